# revision 29
# baseline (speedup 1.0000x reference)
# Multi-head attention (RoPE, causal) Trainium2 Bass kernel.
# B=2, S=2048, D=1024, 16 heads, hd=64, fp32 I/O.
#
# Sharding: 32 (batch, head) units over 8 cores -> each core gets one batch
# and 4 heads. Each core computes its 4 heads' attention output and the
# partial out-projection (sum over its heads); the host sums the 4 partials
# per batch and adds the bias constant.
#
# Self-contained: all shapes/sharding hardcoded; no sibling imports.

import numpy as np

import concourse.bass as bass  # noqa: F401
import concourse.mybir as mybir
import concourse.tile as tile
from concourse import bacc, bass_utils

F32 = mybir.dt.float32
BF16 = mybir.dt.bfloat16
EXP = mybir.ActivationFunctionType.Exp

B = 2
S = 2048
D = 1024
NHEADS = 16
HD = 64
HPC = 4  # heads per core
NCORES = 8
NPAIR = 2  # head pairs per core
P = 128
CH = 512  # q chunk
THETA = 10000.0
QKVW = 3 * HPC * HD  # 768

# module-level knobs for test harness
TRACE = False
LAST_RESULTS = None

_PROGRAM_CACHE = {}


def build_program(s=S, mm_fast=True):
    """Build + compile the single-core SPMD program.

    mm_fast=True: bf16 for all PE operands (fp32 PSUM accumulation).
    mm_fast=False: everything fp32 (4x slower matmuls, reference-grade).
    """
    nt = s // P      # s-tiles
    nch = s // CH    # q chunks
    kt = D // P      # 8 contraction tiles
    PD = BF16 if mm_fast else F32

    nc = bacc.Bacc(
        "TRN2", target_bir_lowering=False, debug=False, enable_asserts=False
    )

    # ---- DRAM I/O ----
    xt_d = nc.dram_tensor("xt", [P, kt * s], PD, kind="ExternalInput").ap()
    wt_d = nc.dram_tensor("wt", [P, kt * QKVW], PD, kind="ExternalInput").ap()
    biasqk_d = nc.dram_tensor("biasqk", [P, 512], PD, kind="ExternalInput").ap()
    ropec_d = nc.dram_tensor("ropec", [P, nt * 256], PD, kind="ExternalInput").ap()
    ropes_d = nc.dram_tensor("ropes", [P, nt * 256], PD, kind="ExternalInput").ap()
    trimask_d = nc.dram_tensor("trimask", [P, P], PD, kind="ExternalInput").ap()
    ident_d = nc.dram_tensor("ident", [P, P], PD, kind="ExternalInput").ap()
    wo_d = nc.dram_tensor("wo", [P, NPAIR * D], PD, kind="ExternalInput").ap()
    out_d = nc.dram_tensor("outp", [s, D], F32, kind="ExternalOutput").ap()

    from contextlib import ExitStack

    with tile.TileContext(nc) as tc, ExitStack() as ctx:
        const = ctx.enter_context(tc.tile_pool(name="const", bufs=1))

        # const loads ride the ACT HWDGE ring so they don't serialize with
        # the x-tile loads on the sync ring
        wt_sb = const.tile([P, kt * QKVW], PD)
        qtr = kt * QKVW // 4
        for qi in range(4):
            nc.scalar.dma_start(
                wt_sb[:, qi * qtr : (qi + 1) * qtr],
                wt_d[:, qi * qtr : (qi + 1) * qtr],
            )
        biasqk_sb = const.tile([P, 512], PD)
        nc.scalar.dma_start(biasqk_sb[:], biasqk_d[:])
        ident_sb = const.tile([P, P], PD)
        nc.scalar.dma_start(ident_sb[:], ident_d[:])
        ropec_sb = const.tile([P, nt * 256], PD)
        nc.scalar.dma_start(ropec_sb[:], ropec_d[:])
        ropes_sb = const.tile([P, nt * 256], PD)
        nc.scalar.dma_start(ropes_sb[:], ropes_d[:])
        trimask_sb = const.tile([P, P], PD)
        nc.scalar.dma_start(trimask_sb[:], trimask_d[:])
        wo_sb = const.tile([P, NPAIR * D], PD)
        nc.scalar.dma_start(wo_sb[:], wo_d[:])

        # persistent activations
        # qkt2 blocks: [Qpack0, Qpack1, Kpack0, Kpack1] each [128 (2 heads*hd), s]
        qkt2 = const.tile([P, 4 * s], PD)
        # vone: per s-tile [128, 4*65]; per head 64 V cols + ones col
        vone = const.tile([P, nt * (HPC * 65)], PD)
        # otn2: O^T packs (normalized in place): [128 (2 heads*hd), s] per pair
        otn2 = const.tile([P, NPAIR * s], PD)

        # ones columns of vone
        vone_v = vone.rearrange("p (t h c) -> p t h c", t=nt, h=HPC)
        nc.gpsimd.memset(vone_v[:, :, :, 64], 1.0)

        # half-selector columns for the rank-1 denominator broadcast:
        # halfsel[0, 0:128] selects partitions 0-63, [128:256] selects 64-127
        halfsel = const.tile([1, 256], PD)
        nc.gpsimd.memset(halfsel[:, 0:64], 1.0)
        nc.gpsimd.memset(halfsel[:, 64:192], 0.0)
        nc.gpsimd.memset(halfsel[:, 192:256], 1.0)

        xt_dv = xt_d.rearrange("p (k s) -> p k s", k=kt)

        # ---------------- Phase A: QKV + RoPE + transposes ----------------
        bpsum = ctx.enter_context(tc.tile_pool(name="bpsum", bufs=2, space="PSUM"))
        awork = ctx.enter_context(tc.tile_pool(name="awork", bufs=3))

        def emit_A(st):
            if True:
                xts = awork.tile([P, D], PD, name="xts")
                nc.sync.dma_start(
                    xts.rearrange("p (k j) -> p k j", k=kt),
                    xt_dv[:, :, st * P : (st + 1) * P],
                )
                psA = bpsum.tile([P, 1024], F32, name="psA", tag="big")
                for k in range(kt):
                    lhs = xts[:, k * P : (k + 1) * P]
                    rhs = wt_sb[:, k * QKVW : k * QKVW + QKVW]
                    nc.tensor.matmul(
                        psA[:, 0:512],
                        lhsT=lhs,
                        rhs=rhs[:, 0:512],
                        start=(k == 0),
                        stop=(k == kt - 1),
                    )
                    nc.tensor.matmul(
                        psA[:, 512:768],
                        lhsT=lhs,
                        rhs=rhs[:, 512:768],
                        start=(k == 0),
                        stop=(k == kt - 1),
                    )
                # evict V into vone slots (no V bias: folded into host const)
                nc.scalar.copy(
                    vone_v[:, st, :, 0:64],
                    psA[:, 512:768].rearrange("p (h c) -> p h c", h=HPC),
                )
                # evict Q,K with bias
                qk = awork.tile([P, 512], PD, name="qk")
                nc.vector.tensor_add(qk[:], psA[:, 0:512], biasqk_sb[:])
                # rope: rot = qk*cos + swap(qk)*sin
                sw = awork.tile([P, 512], PD, name="sw")
                qk_v = qk.rearrange("p (n two) -> p n two", two=2)
                sw_v = sw.rearrange("p (n two) -> p n two", two=2)
                nc.vector.tensor_copy(sw_v[:, :, 0], qk_v[:, :, 1])
                nc.vector.tensor_copy(sw_v[:, :, 1], qk_v[:, :, 0])
                rc = ropec_sb[:, st * 256 : (st + 1) * 256]
                rs = ropes_sb[:, st * 256 : (st + 1) * 256]
                rot = awork.tile([P, 512], PD, name="rot")
                nc.vector.tensor_mul(rot[:, 0:256], qk[:, 0:256], rc)
                nc.vector.tensor_mul(rot[:, 256:512], qk[:, 256:512], rc)
                nc.gpsimd.tensor_mul(sw[:, 0:256], sw[:, 0:256], rs)
                nc.gpsimd.tensor_mul(sw[:, 256:512], sw[:, 256:512], rs)
                nc.vector.tensor_add(rot[:], rot[:], sw[:])
                # transpose the 4 pack blocks -> qkt2
                tp = bpsum.tile([P, 512], PD, name="tp", tag="acc")
                for b in range(4):
                    nc.tensor.transpose(
                        tp[:, b * P : (b + 1) * P],
                        rot[:, b * P : (b + 1) * P],
                        ident_sb[:],
                    )
                nc.scalar.copy(
                    qkt2.rearrange("p (b s) -> p b s", b=4)[
                        :, :, st * P : (st + 1) * P
                    ],
                    tp.rearrange("p (b j) -> p b j", b=4),
                )

        # ---------------- Phase B: attention ----------------
        bwork = ctx.enter_context(tc.tile_pool(name="bwork", bufs=3))
        fxwork = ctx.enter_context(tc.tile_pool(name="fxwork", bufs=2))
        # softmax denominator collection: per (pair, J): [headA 512 | headB 512]
        dall = const.tile([1, NPAIR * nch * 1024], F32)
        dallinv = const.tile([1, NPAIR * nch * 1024], PD)

        def emit_BJ(p, j):
            q_pack = qkt2[:, p * s : (p + 1) * s]
            k_pack = qkt2[:, (2 + p) * s : (3 + p) * s]
            ot2 = bpsum.tile([P, 1024], F32, name="ot2", tag="acc")
            mlast = 4 * j + 3
            for m in range(4 * j + 4):
                sc2 = bpsum.tile([P, 1024], F32, name="sc2", tag="big")
                kA = k_pack[0:64, m * P : (m + 1) * P]
                kB = k_pack[64:128, m * P : (m + 1) * P]
                qA = q_pack[0:64, j * CH : (j + 1) * CH]
                qB = q_pack[64:128, j * CH : (j + 1) * CH]
                nc.tensor.matmul(sc2[:, 0:512], lhsT=kA, rhs=qA)
                nc.tensor.matmul(sc2[:, 512:1024], lhsT=kB, rhs=qB)
                at2 = bwork.tile([P, 1024], PD, name="at2")
                # columns below the diagonal block are dead: skip them in
                # exp and in the AV accumulation entirely
                off = m * P - j * CH if m >= 4 * j else 0
                if off > 0:
                    sc_v = sc2.rearrange("p (h q) -> p h q", h=2)
                    at_v = at2.rearrange("p (h q) -> p h q", h=2)
                    nc.scalar.activation(
                        at_v[:, :, off:512],
                        sc_v[:, :, off:512],
                        EXP,
                        scale=0.125,
                    )
                else:
                    nc.scalar.activation(at2[:], sc2[:], EXP, scale=0.125)
                if m >= 4 * j:
                    nc.vector.tensor_mul(
                        at2[:, off : off + P],
                        at2[:, off : off + P],
                        trimask_sb[:],
                    )
                    nc.vector.tensor_mul(
                        at2[:, 512 + off : 512 + off + P],
                        at2[:, 512 + off : 512 + off + P],
                        trimask_sb[:],
                    )
                vA = vone_v[:, m, 2 * p, :]
                vB = vone_v[:, m, 2 * p + 1, :]
                nc.tensor.matmul(
                    ot2[0:65, off:512],
                    lhsT=vA,
                    rhs=at2[:, off:512],
                    start=(m == 0),
                    stop=(m == mlast),
                )
                nc.tensor.matmul(
                    ot2[0:65, 512 + off : 1024],
                    lhsT=vB,
                    rhs=at2[:, 512 + off : 1024],
                    start=(m == 0),
                    stop=(m == mlast),
                )
            # ---- fixup: evict OT halves + denominators (per J) ----
            nc.vector.tensor_copy(
                otn2[0:64, p * s + j * CH : p * s + (j + 1) * CH],
                ot2[0:64, 0:512],
            )
            stgB = fxwork.tile([64, 512], PD, name="stgB")
            nc.vector.tensor_copy(stgB[:], ot2[0:64, 512:1024])
            nc.sync.dma_start(
                otn2[64:128, p * s + j * CH : p * s + (j + 1) * CH],
                stgB[:],
            )
            dslot = (p * nch + j) * 1024
            nc.vector.tensor_copy(
                dall[0:1, dslot : dslot + 512], ot2[64:65, 0:512]
            )
            nc.vector.tensor_copy(
                dall[0:1, dslot + 512 : dslot + 1024], ot2[64:65, 512:1024]
            )
            # reciprocal in partition-parallel layout: scatter the 1024 D
            # values across partitions, one wide reciprocal, scatter back
            dPj = fxwork.tile([P, 8], F32, name="dPj")
            nc.sync.dma_start(
                dPj[:],
                dall[0:1, dslot : dslot + 1024].rearrange("o (a b) -> o a b", a=P),
            )
            dPq = fxwork.tile([P, 8], F32, name="dPq")
            nc.vector.reciprocal(dPq[:], dPj[:])
            dPc = fxwork.tile([P, 8], PD, name="dPc")
            with nc.allow_low_precision("softmax denominators"):
                nc.vector.tensor_copy(dPc[:], dPq[:])
            nc.sync.dma_start(
                dallinv[0:1, dslot : dslot + 1024].rearrange(
                    "o (a b) -> o a b", a=P
                ),
                dPc[:],
            )
        def emit_final(p, j):
            dslot = (p * nch + j) * 1024
            # broadcast denominators across partitions via rank-1 matmuls
            # (rows 0-63 get head A's dinv, 64-127 head B's) and normalize
            dvb = bpsum.tile([P, 512], F32, name="dvb", tag="acc")
            nc.tensor.matmul(
                dvb[:],
                lhsT=halfsel[0:1, 0:128],
                rhs=dallinv[0:1, dslot : dslot + 512],
                start=True,
                stop=False,
            )
            nc.tensor.matmul(
                dvb[:],
                lhsT=halfsel[0:1, 128:256],
                rhs=dallinv[0:1, dslot + 512 : dslot + 1024],
                start=False,
                stop=True,
            )
            nc.vector.tensor_mul(
                otn2[:, p * s + j * CH : p * s + (j + 1) * CH],
                otn2[:, p * s + j * CH : p * s + (j + 1) * CH],
                dvb[:],
            )


        cwork = ctx.enter_context(tc.tile_pool(name="cwork", bufs=3))

        def emit_C(g):
            # out projection for q-tiles 4g..4g+3 (needs both pairs' chunk-g
            # finals done)
            for qt in range(4 * g, min(4 * g + 4, nt)):
                outsb = cwork.tile([P, D], F32, name="outsb")
                for dc in range(2):
                    pr = bpsum.tile([P, 512], F32, name="pr", tag="big")
                    for p in range(NPAIR):
                        nc.tensor.matmul(
                            pr[:],
                            lhsT=otn2[:, p * s + qt * P : p * s + (qt + 1) * P],
                            rhs=wo_sb[:, p * D + dc * 512 : p * D + (dc + 1) * 512],
                            start=(p == 0),
                            stop=(p == NPAIR - 1),
                        )
                    if dc == 0:
                        nc.vector.tensor_copy(outsb[:, 0:512], pr[:])
                    else:
                        nc.scalar.copy(outsb[:, 512:1024], pr[:])
                nc.sync.dma_start(out_d[qt * P : (qt + 1) * P, :], outsb[:])

        # sequential A then B; fixup finals pipeline one chunk behind so the
        # rank-1 broadcast + normalize never gate the next chunk's PSUM; the
        # out-projection interleaves per chunk as soon as both pairs' finals
        # for that q-range are emitted
        for st in range(nt):
            emit_A(st)
        pending = None
        for p in range(NPAIR):
            for j in range(nch):
                emit_BJ(p, j)
                if pending is not None:
                    emit_final(*pending)
                    if pending[0] == 1:
                        emit_C(pending[1])
                pending = (p, j)
        emit_final(*pending)
        emit_C(pending[1])

    nc.compile()
    return nc


def get_program(s=S, mm_fast=True):
    key = (s, mm_fast)
    if key not in _PROGRAM_CACHE:
        _PROGRAM_CACHE[key] = build_program(s, mm_fast)
    return _PROGRAM_CACHE[key]


def _to_pd(a, mm_fast):
    if mm_fast:
        import ml_dtypes

        return np.ascontiguousarray(a).astype(ml_dtypes.bfloat16)
    return np.ascontiguousarray(a).astype(np.float32)


def prep_core_inputs(x, w_qkv, b_qkv, w_out, core, s=S, mm_fast=True):
    """Build the per-core input map (numpy, host-side sharding/layout)."""
    nt = s // P
    kt = D // P
    b = core // 4
    heads = [(core % 4) * HPC + i for i in range(HPC)]

    xb = np.ascontiguousarray(x[b][:s])  # [s, D]
    # xt[p, k*s + j] = x[j, k*128+p]
    xt = np.ascontiguousarray(
        xb.reshape(s, kt, P).transpose(2, 1, 0).reshape(P, kt * s)
    )

    rows = []
    for part in range(3):
        for h in heads:
            rows.extend(range(part * D + h * HD, part * D + (h + 1) * HD))
    w_sel = w_qkv[rows]  # [768, 1024]
    b_sel = b_qkv[rows]  # [768]
    # wt[p, k*768 + n] = w_sel[n, k*128+p]
    wt = np.ascontiguousarray(
        w_sel.T.reshape(kt, P, QKVW).transpose(1, 0, 2).reshape(P, kt * QKVW)
    )
    biasqk = np.broadcast_to(b_sel[None, 0:512], (P, 512)).copy()

    # rope tables, natural layout per s-tile: [p, st*256 + jj]
    dims = np.arange(0, HD, 2, dtype=np.float64)
    invf = 1.0 / (THETA ** (dims / HD))  # [32]
    pos = np.arange(s, dtype=np.float64)
    ang = pos[:, None] * invf[None, :]  # [s, 32]
    c = np.cos(ang)
    sn = np.sin(ang)
    c2 = np.repeat(c, 2, axis=1)  # [s, 64]
    s2 = np.empty((s, HD))
    s2[:, 0::2] = -sn
    s2[:, 1::2] = sn
    c2h = np.tile(c2, (1, HPC))  # [s, 256]
    s2h = np.tile(s2, (1, HPC))
    ropec = np.ascontiguousarray(
        c2h.reshape(nt, P, 256).transpose(1, 0, 2).reshape(P, nt * 256)
    )
    ropes = np.ascontiguousarray(
        s2h.reshape(nt, P, 256).transpose(1, 0, 2).reshape(P, nt * 256)
    )

    trimask = np.triu(np.ones((P, P), dtype=np.float32))
    ident = np.eye(P, dtype=np.float32)

    # wo[kk, p2*D + n] = w_out[n, gh*64 + kk%64], gh = heads[2*p2 + kk//64]
    wo = np.empty((P, NPAIR * D), dtype=np.float32)
    for p2 in range(NPAIR):
        for half in range(2):
            gh = heads[2 * p2 + half]
            wo[half * 64 : (half + 1) * 64, p2 * D : (p2 + 1) * D] = w_out[
                :, gh * HD : (gh + 1) * HD
            ].T
    return {
        "xt": _to_pd(xt, mm_fast),
        "wt": _to_pd(wt, mm_fast),
        "biasqk": _to_pd(biasqk, mm_fast),
        "ropec": _to_pd(ropec, mm_fast),
        "ropes": _to_pd(ropes, mm_fast),
        "trimask": _to_pd(trimask, mm_fast),
        "ident": _to_pd(ident, mm_fast),
        "wo": _to_pd(wo, mm_fast),
    }


def kernel(x, w_qkv, b_qkv, w_out, b_out, mm_fast=True):
    global LAST_RESULTS
    x = np.asarray(x, dtype=np.float32)
    w_qkv = np.asarray(w_qkv, dtype=np.float32)
    b_qkv = np.asarray(b_qkv, dtype=np.float32)
    w_out = np.asarray(w_out, dtype=np.float32)
    b_out = np.asarray(b_out, dtype=np.float32)

    nc = get_program(mm_fast=mm_fast)
    in_maps = [
        prep_core_inputs(x, w_qkv, b_qkv, w_out, core, mm_fast=mm_fast)
        for core in range(NCORES)
    ]
    res = bass_utils.run_bass_kernel_spmd(
        nc, in_maps, core_ids=list(range(NCORES)), trace=TRACE
    )
    LAST_RESULTS = res
    partials = [r["outp"] for r in res.results]
    # v-bias contribution is constant across s (sum_k attn = 1):
    bconst = b_out + b_qkv[2 * D : 3 * D] @ w_out.T
    out = np.stack(
        [
            partials[0] + partials[1] + partials[2] + partials[3],
            partials[4] + partials[5] + partials[6] + partials[7],
        ]
    )
    out = out + bconst[None, None, :]
    return out.astype(np.float32)


# revision 31
# speedup vs baseline: 1.0140x; 1.0140x over previous
# Multi-head attention (RoPE, causal) Trainium2 Bass kernel.
# B=2, S=2048, D=1024, 16 heads, hd=64, fp32 I/O.
#
# Sharding: 32 (batch, head) units over 8 cores -> each core gets one batch
# and 4 heads. Each core computes its 4 heads' attention output and the
# partial out-projection (sum over its heads); the host sums the 4 partials
# per batch and adds the bias constant.
#
# Self-contained: all shapes/sharding hardcoded; no sibling imports.

import numpy as np

import concourse.bass as bass  # noqa: F401
import concourse.mybir as mybir
import concourse.tile as tile
from concourse import bacc, bass_utils

F32 = mybir.dt.float32
BF16 = mybir.dt.bfloat16
EXP = mybir.ActivationFunctionType.Exp

B = 2
S = 2048
D = 1024
NHEADS = 16
HD = 64
HPC = 4  # heads per core
NCORES = 8
NPAIR = 2  # head pairs per core
P = 128
CH = 512  # q chunk
THETA = 10000.0
QKVW = 3 * HPC * HD  # 768

# module-level knobs for test harness
TRACE = False
LAST_RESULTS = None

_PROGRAM_CACHE = {}


def build_program(s=S, mm_fast=True):
    """Build + compile the single-core SPMD program.

    mm_fast=True: bf16 for all PE operands (fp32 PSUM accumulation).
    mm_fast=False: everything fp32 (4x slower matmuls, reference-grade).
    """
    nt = s // P      # s-tiles
    nch = s // CH    # q chunks
    kt = D // P      # 8 contraction tiles
    PD = BF16 if mm_fast else F32

    nc = bacc.Bacc(
        "TRN2", target_bir_lowering=False, debug=False, enable_asserts=False
    )

    # ---- DRAM I/O ----
    xt_d = nc.dram_tensor("xt", [P, kt * s], PD, kind="ExternalInput").ap()
    wt_d = nc.dram_tensor("wt", [P, kt * QKVW], PD, kind="ExternalInput").ap()
    biasqk_d = nc.dram_tensor("biasqk", [P, 512], PD, kind="ExternalInput").ap()
    ropec_d = nc.dram_tensor("ropec", [P, nt * 256], PD, kind="ExternalInput").ap()
    ropes_d = nc.dram_tensor("ropes", [P, nt * 256], PD, kind="ExternalInput").ap()
    trimask_d = nc.dram_tensor("trimask", [P, P], PD, kind="ExternalInput").ap()
    ident_d = nc.dram_tensor("ident", [P, P], PD, kind="ExternalInput").ap()
    wo_d = nc.dram_tensor("wo", [P, NPAIR * D], PD, kind="ExternalInput").ap()
    out_d = nc.dram_tensor("outp", [s, D], F32, kind="ExternalOutput").ap()

    from contextlib import ExitStack

    with tile.TileContext(nc) as tc, ExitStack() as ctx:
        const = ctx.enter_context(tc.tile_pool(name="const", bufs=1))

        # const loads ride the ACT HWDGE ring so they don't serialize with
        # the x-tile loads on the sync ring
        wt_sb = const.tile([P, kt * QKVW], PD)
        qtr = kt * QKVW // 4
        for qi in range(4):
            nc.scalar.dma_start(
                wt_sb[:, qi * qtr : (qi + 1) * qtr],
                wt_d[:, qi * qtr : (qi + 1) * qtr],
            )
        biasqk_sb = const.tile([P, 512], PD)
        nc.scalar.dma_start(biasqk_sb[:], biasqk_d[:])
        ident_sb = const.tile([P, P], PD)
        nc.scalar.dma_start(ident_sb[:], ident_d[:])
        ropec_sb = const.tile([P, nt * 256], PD)
        nc.scalar.dma_start(ropec_sb[:], ropec_d[:])
        ropes_sb = const.tile([P, nt * 256], PD)
        nc.scalar.dma_start(ropes_sb[:], ropes_d[:])
        trimask_sb = const.tile([P, P], PD)
        nc.scalar.dma_start(trimask_sb[:], trimask_d[:])
        wo_sb = const.tile([P, NPAIR * D], PD)
        nc.scalar.dma_start(wo_sb[:], wo_d[:])

        # persistent activations
        # qkt2 blocks: [Qpack0, Qpack1, Kpack0, Kpack1] each [128 (2 heads*hd), s]
        qkt2 = const.tile([P, 4 * s], PD)
        # vone: per s-tile [128, 4*65]; per head 64 V cols + ones col
        vone = const.tile([P, nt * (HPC * 65)], PD)
        # otn2: O^T packs (normalized in place): [128 (2 heads*hd), s] per pair
        otn2 = const.tile([P, NPAIR * s], PD)

        # ones columns of vone
        vone_v = vone.rearrange("p (t h c) -> p t h c", t=nt, h=HPC)
        nc.gpsimd.memset(vone_v[:, :, :, 64], 1.0)

        # half-selector columns for the rank-1 denominator broadcast:
        # halfsel[0, 0:128] selects partitions 0-63, [128:256] selects 64-127
        halfsel = const.tile([1, 256], PD)
        nc.gpsimd.memset(halfsel[:, 0:64], 1.0)
        nc.gpsimd.memset(halfsel[:, 64:192], 0.0)
        nc.gpsimd.memset(halfsel[:, 192:256], 1.0)

        xt_dv = xt_d.rearrange("p (k s) -> p k s", k=kt)

        # ---------------- Phase A: QKV + RoPE + transposes ----------------
        bpsum = ctx.enter_context(tc.tile_pool(name="bpsum", bufs=2, space="PSUM"))
        awork = ctx.enter_context(tc.tile_pool(name="awork", bufs=3))

        def emit_A(st):
            if True:
                xts = awork.tile([P, D], PD, name="xts")
                nc.sync.dma_start(
                    xts.rearrange("p (k j) -> p k j", k=kt),
                    xt_dv[:, :, st * P : (st + 1) * P],
                )
                psA = bpsum.tile([P, 1024], F32, name="psA", tag="big")
                for k in range(kt):
                    lhs = xts[:, k * P : (k + 1) * P]
                    rhs = wt_sb[:, k * QKVW : k * QKVW + QKVW]
                    nc.tensor.matmul(
                        psA[:, 0:512],
                        lhsT=lhs,
                        rhs=rhs[:, 0:512],
                        start=(k == 0),
                        stop=(k == kt - 1),
                    )
                    nc.tensor.matmul(
                        psA[:, 512:768],
                        lhsT=lhs,
                        rhs=rhs[:, 512:768],
                        start=(k == 0),
                        stop=(k == kt - 1),
                    )
                # evict V into vone slots (no V bias: folded into host const)
                nc.scalar.copy(
                    vone_v[:, st, :, 0:64],
                    psA[:, 512:768].rearrange("p (h c) -> p h c", h=HPC),
                )
                # evict Q,K with bias
                qk = awork.tile([P, 512], PD, name="qk")
                nc.vector.tensor_add(qk[:], psA[:, 0:512], biasqk_sb[:])
                # rope: rot = qk*cos + swap(qk)*sin
                sw = awork.tile([P, 512], PD, name="sw")
                qk_v = qk.rearrange("p (n two) -> p n two", two=2)
                sw_v = sw.rearrange("p (n two) -> p n two", two=2)
                nc.vector.tensor_copy(sw_v[:, :, 0], qk_v[:, :, 1])
                nc.vector.tensor_copy(sw_v[:, :, 1], qk_v[:, :, 0])
                rc = ropec_sb[:, st * 256 : (st + 1) * 256]
                rs = ropes_sb[:, st * 256 : (st + 1) * 256]
                rot = awork.tile([P, 512], PD, name="rot")
                nc.vector.tensor_mul(rot[:, 0:256], qk[:, 0:256], rc)
                nc.vector.tensor_mul(rot[:, 256:512], qk[:, 256:512], rc)
                nc.gpsimd.tensor_mul(sw[:, 0:256], sw[:, 0:256], rs)
                nc.gpsimd.tensor_mul(sw[:, 256:512], sw[:, 256:512], rs)
                nc.vector.tensor_add(rot[:], rot[:], sw[:])
                # transpose the 4 pack blocks -> qkt2
                tp = bpsum.tile([P, 512], PD, name="tp", tag="acc")
                for b in range(4):
                    nc.tensor.transpose(
                        tp[:, b * P : (b + 1) * P],
                        rot[:, b * P : (b + 1) * P],
                        ident_sb[:],
                    )
                nc.scalar.copy(
                    qkt2.rearrange("p (b s) -> p b s", b=4)[
                        :, :, st * P : (st + 1) * P
                    ],
                    tp.rearrange("p (b j) -> p b j", b=4),
                )

        # ---------------- Phase B: attention ----------------
        bwork = ctx.enter_context(tc.tile_pool(name="bwork", bufs=3))
        fxwork = ctx.enter_context(tc.tile_pool(name="fxwork", bufs=2))
        # softmax denominator collection: per (pair, J): [headA 512 | headB 512]
        dall = const.tile([1, NPAIR * nch * 1024], F32)
        dallinv = const.tile([1, NPAIR * nch * 1024], PD)

        def emit_BJ(p, j):
            q_pack = qkt2[:, p * s : (p + 1) * s]
            k_pack = qkt2[:, (2 + p) * s : (3 + p) * s]
            ot2 = bpsum.tile([P, 1024], F32, name="ot2", tag="acc")
            mlast = 4 * j + 3
            for m in range(4 * j + 4):
                sc2 = bpsum.tile([P, 1024], F32, name="sc2", tag="big")
                kA = k_pack[0:64, m * P : (m + 1) * P]
                kB = k_pack[64:128, m * P : (m + 1) * P]
                qA = q_pack[0:64, j * CH : (j + 1) * CH]
                qB = q_pack[64:128, j * CH : (j + 1) * CH]
                nc.tensor.matmul(sc2[:, 0:512], lhsT=kA, rhs=qA)
                nc.tensor.matmul(sc2[:, 512:1024], lhsT=kB, rhs=qB)
                at2 = bwork.tile([P, 1024], PD, name="at2")
                # columns below the diagonal block are dead: skip them in
                # exp and in the AV accumulation entirely
                off = m * P - j * CH if m >= 4 * j else 0
                if off > 0:
                    sc_v = sc2.rearrange("p (h q) -> p h q", h=2)
                    at_v = at2.rearrange("p (h q) -> p h q", h=2)
                    nc.scalar.activation(
                        at_v[:, :, off:512],
                        sc_v[:, :, off:512],
                        EXP,
                        scale=0.125,
                    )
                else:
                    nc.scalar.activation(at2[:], sc2[:], EXP, scale=0.125)
                if m >= 4 * j:
                    nc.vector.tensor_mul(
                        at2[:, off : off + P],
                        at2[:, off : off + P],
                        trimask_sb[:],
                    )
                    nc.vector.tensor_mul(
                        at2[:, 512 + off : 512 + off + P],
                        at2[:, 512 + off : 512 + off + P],
                        trimask_sb[:],
                    )
                vA = vone_v[:, m, 2 * p, :]
                vB = vone_v[:, m, 2 * p + 1, :]
                nc.tensor.matmul(
                    ot2[0:65, off:512],
                    lhsT=vA,
                    rhs=at2[:, off:512],
                    start=(m == 0),
                    stop=(m == mlast),
                )
                nc.tensor.matmul(
                    ot2[0:65, 512 + off : 1024],
                    lhsT=vB,
                    rhs=at2[:, 512 + off : 1024],
                    start=(m == 0),
                    stop=(m == mlast),
                )
            # ---- fixup: evict OT halves + denominators (per J) ----
            nc.vector.tensor_copy(
                otn2[0:64, p * s + j * CH : p * s + (j + 1) * CH],
                ot2[0:64, 0:512],
            )
            stgB = fxwork.tile([64, 512], PD, name="stgB")
            nc.vector.tensor_copy(stgB[:], ot2[0:64, 512:1024])
            nc.sync.dma_start(
                otn2[64:128, p * s + j * CH : p * s + (j + 1) * CH],
                stgB[:],
            )
            dslot = (p * nch + j) * 1024
            nc.vector.tensor_copy(
                dall[0:1, dslot : dslot + 512], ot2[64:65, 0:512]
            )
            nc.vector.tensor_copy(
                dall[0:1, dslot + 512 : dslot + 1024], ot2[64:65, 512:1024]
            )
            # reciprocal in partition-parallel layout: scatter the 1024 D
            # values across partitions, one wide reciprocal, scatter back
            dPj = fxwork.tile([P, 8], F32, name="dPj")
            nc.sync.dma_start(
                dPj[:],
                dall[0:1, dslot : dslot + 1024].rearrange("o (a b) -> o a b", a=P),
            )
            dPq = fxwork.tile([P, 8], F32, name="dPq")
            nc.vector.reciprocal(dPq[:], dPj[:])
            dPc = fxwork.tile([P, 8], PD, name="dPc")
            with nc.allow_low_precision("softmax denominators"):
                nc.vector.tensor_copy(dPc[:], dPq[:])
            nc.sync.dma_start(
                dallinv[0:1, dslot : dslot + 1024].rearrange(
                    "o (a b) -> o a b", a=P
                ),
                dPc[:],
            )
        def emit_final(p, j):
            dslot = (p * nch + j) * 1024
            # broadcast denominators across partitions via rank-1 matmuls
            # (rows 0-63 get head A's dinv, 64-127 head B's) and normalize
            dvb = bpsum.tile([P, 512], F32, name="dvb", tag="acc")
            nc.tensor.matmul(
                dvb[:],
                lhsT=halfsel[0:1, 0:128],
                rhs=dallinv[0:1, dslot : dslot + 512],
                start=True,
                stop=False,
            )
            nc.tensor.matmul(
                dvb[:],
                lhsT=halfsel[0:1, 128:256],
                rhs=dallinv[0:1, dslot + 512 : dslot + 1024],
                start=False,
                stop=True,
            )
            nc.vector.tensor_mul(
                otn2[:, p * s + j * CH : p * s + (j + 1) * CH],
                otn2[:, p * s + j * CH : p * s + (j + 1) * CH],
                dvb[:],
            )


        cwork = ctx.enter_context(tc.tile_pool(name="cwork", bufs=3))

        def emit_C(g):
            # out projection for q-tiles 4g..4g+3 (needs both pairs' chunk-g
            # finals done)
            for qt in range(4 * g, min(4 * g + 4, nt)):
                outsb = cwork.tile([P, D], F32, name="outsb")
                for dc in range(2):
                    pr = bpsum.tile([P, 512], F32, name="pr", tag="big")
                    for p in range(NPAIR):
                        nc.tensor.matmul(
                            pr[:],
                            lhsT=otn2[:, p * s + qt * P : p * s + (qt + 1) * P],
                            rhs=wo_sb[:, p * D + dc * 512 : p * D + (dc + 1) * 512],
                            start=(p == 0),
                            stop=(p == NPAIR - 1),
                        )
                    if dc == 0:
                        nc.vector.tensor_copy(outsb[:, 0:512], pr[:])
                    else:
                        nc.scalar.copy(outsb[:, 512:1024], pr[:])
                nc.sync.dma_start(out_d[qt * P : (qt + 1) * P, :], outsb[:])

        # sequential A then B; fixup finals pipeline one chunk behind so the
        # rank-1 broadcast + normalize never gate the next chunk's PSUM; the
        # out-projection interleaves per chunk as soon as both pairs' finals
        # for that q-range are emitted
        for st in range(nt):
            emit_A(st)
        pending = None
        for p in range(NPAIR):
            for j in range(nch):
                emit_BJ(p, j)
                if pending is not None:
                    emit_final(*pending)
                    if pending[0] == 1:
                        emit_C(pending[1])
                pending = (p, j)
        emit_final(*pending)
        emit_C(pending[1])

    nc.compile()
    return nc


def get_program(s=S, mm_fast=True):
    key = (s, mm_fast)
    if key not in _PROGRAM_CACHE:
        _PROGRAM_CACHE[key] = build_program(s, mm_fast)
    return _PROGRAM_CACHE[key]


def _to_pd(a, mm_fast):
    if mm_fast:
        import ml_dtypes

        return np.ascontiguousarray(a).astype(ml_dtypes.bfloat16)
    return np.ascontiguousarray(a).astype(np.float32)


def prep_core_inputs(x, w_qkv, b_qkv, w_out, core, s=S, mm_fast=True):
    """Build the per-core input map (numpy, host-side sharding/layout)."""
    nt = s // P
    kt = D // P
    b = core // 4
    heads = [(core % 4) * HPC + i for i in range(HPC)]

    xb = np.ascontiguousarray(x[b][:s])  # [s, D]
    # xt[p, k*s + j] = x[j, k*128+p]
    xt = np.ascontiguousarray(
        xb.reshape(s, kt, P).transpose(2, 1, 0).reshape(P, kt * s)
    )

    rows = []
    for part in range(3):
        for h in heads:
            rows.extend(range(part * D + h * HD, part * D + (h + 1) * HD))
    w_sel = w_qkv[rows]  # [768, 1024]
    b_sel = b_qkv[rows]  # [768]
    # wt[p, k*768 + n] = w_sel[n, k*128+p]
    wt = np.ascontiguousarray(
        w_sel.T.reshape(kt, P, QKVW).transpose(1, 0, 2).reshape(P, kt * QKVW)
    )
    biasqk = np.broadcast_to(b_sel[None, 0:512], (P, 512)).copy()

    # rope tables, natural layout per s-tile: [p, st*256 + jj]
    dims = np.arange(0, HD, 2, dtype=np.float64)
    invf = 1.0 / (THETA ** (dims / HD))  # [32]
    pos = np.arange(s, dtype=np.float64)
    ang = pos[:, None] * invf[None, :]  # [s, 32]
    c = np.cos(ang)
    sn = np.sin(ang)
    c2 = np.repeat(c, 2, axis=1)  # [s, 64]
    s2 = np.empty((s, HD))
    s2[:, 0::2] = -sn
    s2[:, 1::2] = sn
    c2h = np.tile(c2, (1, HPC))  # [s, 256]
    s2h = np.tile(s2, (1, HPC))
    ropec = np.ascontiguousarray(
        c2h.reshape(nt, P, 256).transpose(1, 0, 2).reshape(P, nt * 256)
    )
    ropes = np.ascontiguousarray(
        s2h.reshape(nt, P, 256).transpose(1, 0, 2).reshape(P, nt * 256)
    )

    trimask = np.triu(np.ones((P, P), dtype=np.float32))
    ident = np.eye(P, dtype=np.float32)

    # wo[kk, p2*D + n] = w_out[n, gh*64 + kk%64], gh = heads[2*p2 + kk//64]
    wo = np.empty((P, NPAIR * D), dtype=np.float32)
    for p2 in range(NPAIR):
        for half in range(2):
            gh = heads[2 * p2 + half]
            wo[half * 64 : (half + 1) * 64, p2 * D : (p2 + 1) * D] = w_out[
                :, gh * HD : (gh + 1) * HD
            ].T
    return {
        "xt": _to_pd(xt, mm_fast),
        "wt": _to_pd(wt, mm_fast),
        "biasqk": _to_pd(biasqk, mm_fast),
        "ropec": _to_pd(ropec, mm_fast),
        "ropes": _to_pd(ropes, mm_fast),
        "trimask": _to_pd(trimask, mm_fast),
        "ident": _to_pd(ident, mm_fast),
        "wo": _to_pd(wo, mm_fast),
    }


def kernel(x, w_qkv, b_qkv, w_out, b_out, mm_fast=True):
    global LAST_RESULTS
    x = np.asarray(x, dtype=np.float32)
    w_qkv = np.asarray(w_qkv, dtype=np.float32)
    b_qkv = np.asarray(b_qkv, dtype=np.float32)
    w_out = np.asarray(w_out, dtype=np.float32)
    b_out = np.asarray(b_out, dtype=np.float32)

    nc = get_program(mm_fast=mm_fast)
    in_maps = [
        prep_core_inputs(x, w_qkv, b_qkv, w_out, core, mm_fast=mm_fast)
        for core in range(NCORES)
    ]
    res = bass_utils.run_bass_kernel_spmd(
        nc, in_maps, core_ids=list(range(NCORES)), trace=TRACE
    )
    LAST_RESULTS = res
    partials = [r["outp"] for r in res.results]
    # v-bias contribution is constant across s (sum_k attn = 1):
    bconst = b_out + b_qkv[2 * D : 3 * D] @ w_out.T
    out = np.stack(
        [
            partials[0] + partials[1] + partials[2] + partials[3],
            partials[4] + partials[5] + partials[6] + partials[7],
        ]
    )
    out = out + bconst[None, None, :]
    return out.astype(np.float32)
